# revision 13
# baseline (speedup 1.0000x reference)
"""Trainium2 Bass kernel for NeuralODEMemory (nn_NeuralODEMemory_28355374088720).

Math (reference):
    dt = 0.1, 10 Euler steps over h (N=65536 rows, D=512):
        z = [h, t]                              # time feature column
        deriv = tanh(tanh(z @ W1.T + b1) @ W2.T + b2)
        h <- h + dt * deriv
    gate  = sigmoid([x, h10] @ Wg.T + bg)
    out   = gate * h10 + (1 - gate) * x

Kernel strategy:
  * Data-parallel over 8 NeuronCores (8192 rows each); weights replicated.
  * Feature-major ("transposed") activation layout [D, rows] on chip so the
    per-step matmul chain needs no transposes: weights are the stationary
    operand ([in,out] chunks), activations stream as the moving operand, and
    each layer's PSUM output is already in the layout the next layer consumes.
  * The time-feature column is algebraically folded into a per-step bias:
    z @ W1.T = h @ W1[:, :D].T + t * W1[:, D], so b1_eff(s) = b1 + t_s*W1[:,D].
    Bias-add (and input scaling) is free via ACT: out = f(in*scale + bias).
  * h is kept "primed" as h' = h/dt with W1/WgB pre-scaled by dt on the host,
    so the per-step update is a single tensor_tensor add: h' += deriv.
  * MODE="fp8": matmuls in float8-e4m3 with DoubleRow perf mode (2 fp8
    weights/cell -> K=256 per pass, ~1.77x the bf16 MM stream). Power-of-2
    scale factors keep operands in e4m3's normal range and are undone exactly
    by the ACT input scale. h' accumulator stays fp32, deriv stays bf16,
    x/out stay fp32.
  * MODE="bf16": plain bf16 matmuls (fp32 PSUM accumulation).
  * Host does the cheap prep: weight transpose/scale/cast, x/h transposes.
"""

import os
from contextlib import ExitStack

import numpy as np
import ml_dtypes

N_TOTAL = 65536
D = 512
NCORES = 8
NPC = N_TOTAL // NCORES          # rows per core
NUM_STEPS = 10
TIME_INTERVAL = 1.0
DT = TIME_INTERVAL / NUM_STEPS
P = 128
FK = D // P                      # feature chunks of 128 (4)
MMN = 512                        # matmul moving-operand free dim (one PSUM bank)

MODE = os.environ.get("NODE_KERNEL_MODE", "mix")

# fp8 (e4m3) power-of-2 scale factors
AH = 4.0                         # h' mirror scale
AW1 = 2.0 ** 15                  # on dt*W1s  (|dt*W1s| <= 0.0044 -> <= 145)
AW2 = 2.0 ** 12                  # on W2      (|W2| <= 0.0442 -> <= 181)
AX = 32.0                        # x mirror   (|x| <= ~5.3 -> <= 170)
CG = 2.0 ** 16                   # common gate scale: awga*AX = awgb*AH = CG
AWGA = CG / AX                   # 2048  (|WgA| <= 0.0313 -> <= 64)
AWGB = CG / AH                   # 16384 (|dt*WgB| <= 0.0032 -> <= 52)

_CACHE = {}
LAST = {}                        # stash of the last run's BassKernelResults


def _block_schedule(npc, rblk):
    # Uniform blocks: lead blocks smaller than rblk were tried to shorten the
    # serial head, but 512-row blocks starve the PE mid-kernel (PSUM slots and
    # short matmul groups can't hide the ACT/DVE tails) — a net loss.
    return [(i * rblk, rblk) for i in range(npc // rblk)]


def _build(npc, rblk=1024, mode=MODE):
    import concourse.mybir as mybir
    import concourse.tile as tile
    from concourse import bacc

    f32 = mybir.dt.float32
    bf16 = mybir.dt.bfloat16
    fp8 = mybir.dt.float8e4
    Act = mybir.ActivationFunctionType
    Alu = mybir.AluOpType
    DR = mybir.MatmulPerfMode.DoubleRow

    # per-tensor matmul dtypes: "fp8" = everything, "mix" = both ODE-loop
    # layers fp8 but gate in bf16 (fp8 on the gate path alone costs ~2.5e-2
    # rel err — over the 2e-2 budget), "fp8l2" = only layer 2, else bf16
    cdt = fp8 if mode == "fp8" else bf16          # x-mirror / Wg (gate operands)
    c1dt = fp8 if mode in ("fp8", "mix") else bf16     # h-mirror / W1
    l2dt = fp8 if mode in ("fp8", "fp8l2", "mix") else bf16   # a / W2
    dr1 = mode in ("fp8", "mix")                   # DoubleRow on layer 1
    dr2 = mode in ("fp8", "fp8l2", "mix")          # DoubleRow on layer 2
    drg = mode == "fp8"                            # DoubleRow on gate
    blocks = _block_schedule(npc, rblk)

    nc = bacc.Bacc("TRN2", target_bir_lowering=False, debug=False,
                   num_devices=NCORES)

    hT = nc.dram_tensor("hT", [D, npc], f32, kind="ExternalInput").ap()
    xT = nc.dram_tensor("xT", [D, npc], f32, kind="ExternalInput").ap()
    xTb = nc.dram_tensor("xTb", [D, npc], cdt, kind="ExternalInput").ap()
    w1t = nc.dram_tensor("w1t", [D, D], c1dt, kind="ExternalInput").ap()
    w2t = nc.dram_tensor("w2t", [D, D], l2dt, kind="ExternalInput").ap()
    wgt = nc.dram_tensor("wgt", [2 * D, D], cdt, kind="ExternalInput").ap()
    b1e = nc.dram_tensor("b1e", [P, NUM_STEPS * FK], f32, kind="ExternalInput").ap()
    b2c = nc.dram_tensor("b2c", [P, FK], f32, kind="ExternalInput").ap()
    bgc = nc.dram_tensor("bgc", [P, FK], f32, kind="ExternalInput").ap()
    outT = nc.dram_tensor("outT", [D, npc], f32, kind="ExternalOutput").ap()

    hTr = hT.rearrange("(k p) r -> p k r", p=P)
    xTr = xT.rearrange("(k p) r -> p k r", p=P)
    xTbr = xTb.rearrange("(k p) r -> p k r", p=P)
    outTr = outT.rearrange("(k p) r -> p k r", p=P)

    # ACT input scales that undo the fp8 operand scaling exactly
    s_l1 = 1.0 / (AW1 * AH) if dr1 else 1.0
    s_l2 = 1.0 / AW2 if dr2 else 1.0
    s_g = 1.0 / CG if drg else 1.0

    with tile.TileContext(nc) as tc:
        with ExitStack() as ctx:
            consts = ctx.enter_context(tc.tile_pool(name="consts", bufs=1))
            hp = ctx.enter_context(tc.tile_pool(name="h", bufs=3))
            hbp = ctx.enter_context(tc.tile_pool(name="hb", bufs=2))
            hbgp = (ctx.enter_context(tc.tile_pool(name="hbg", bufs=2))
                    if mode == "mix" else None)
            apool = ctx.enter_context(tc.tile_pool(name="a", bufs=2))
            dpool = ctx.enter_context(tc.tile_pool(name="d", bufs=2))
            xbp = ctx.enter_context(tc.tile_pool(name="xb", bufs=2))
            xfp = ctx.enter_context(tc.tile_pool(name="xf", bufs=2))
            gp = ctx.enter_context(tc.tile_pool(name="g", bufs=2))
            scp = ctx.enter_context(tc.tile_pool(name="sc", bufs=6))
            psp = ctx.enter_context(tc.tile_pool(name="ps", bufs=4, space="PSUM"))

            # Replicated constants, resident for the whole kernel.
            w1 = consts.tile([P, FK, D], c1dt)
            nc.sync.dma_start(w1[:], w1t.rearrange("(k p) m -> p k m", p=P))
            w2 = consts.tile([P, FK, D], l2dt)
            nc.sync.dma_start(w2[:], w2t.rearrange("(k p) m -> p k m", p=P))
            wg = consts.tile([P, 2 * FK, D], cdt)
            nc.sync.dma_start(wg[:], wgt.rearrange("(k p) m -> p k m", p=P))
            b1 = consts.tile([P, NUM_STEPS * FK], f32)
            nc.sync.dma_start(b1[:], b1e)
            b2 = consts.tile([P, FK], f32)
            nc.sync.dma_start(b2[:], b2c)
            bg = consts.tile([P, FK], f32)
            nc.sync.dma_start(bg[:], bgc)

            def mm_group(ps_tile, wtile, wk, m, rhs_tile, rk, start, stop,
                         nsub, dr):
                for sub in range(nsub):
                    if dr:
                        # DoubleRow: one matmul contracts a pair of k-chunks
                        nc.tensor.matmul(
                            ps_tile[:, sub * MMN:(sub + 1) * MMN],
                            wtile[:, 2 * wk:2 * wk + 2, m * P:(m + 1) * P],
                            rhs_tile[:, 2 * rk:2 * rk + 2,
                                     sub * MMN:(sub + 1) * MMN],
                            start=start, stop=stop, perf_mode=DR,
                        )
                    else:
                        nc.tensor.matmul(
                            ps_tile[:, sub * MMN:(sub + 1) * MMN],
                            wtile[:, wk, m * P:(m + 1) * P],
                            rhs_tile[:, rk, sub * MMN:(sub + 1) * MMN],
                            start=start, stop=stop,
                        )

            NK1 = FK // 2 if dr1 else FK      # layer-1 k-group count
            NK2 = FK // 2 if dr2 else FK      # layer-2 k-group count
            NKG = FK // 2 if drg else FK      # gate k-group count (per half)

            def mirror(dst, src):
                # bf16/fp8 matmul mirror of the fp32 h' accumulator
                if mode in ("fp8", "mix"):
                    nc.vector.tensor_scalar_mul(dst, src, AH)
                else:
                    nc.vector.tensor_copy(dst, src)

            for rs, rblk_b in blocks:
                nsub = rblk_b // MMN
                h = hp.tile([P, FK, rblk_b], f32, tag="h")
                # per-chunk loads so each mirror cast starts as soon as its
                # chunk lands (matters for the first block's serial head)
                for k in range(FK):
                    nc.sync.dma_start(h[:, k], hTr[:, k, rs:rs + rblk_b])
                xb = xbp.tile([P, FK, rblk_b], cdt, tag="xb")
                nc.sync.dma_start(xb[:], xTbr[:, :, rs:rs + rblk_b])
                xf = xfp.tile([P, FK, rblk_b], f32, tag="xf")
                nc.sync.dma_start(xf[:], xTr[:, :, rs:rs + rblk_b])

                hb = hbp.tile([P, FK, rblk_b], c1dt, tag="hb")
                for k in range(FK):
                    mirror(hb[:, k], h[:, k])

                for s in range(NUM_STEPS):
                    # layer 1: a = tanh((dt*W1s).T-chunks @ h'b + b1_eff(s))
                    a = apool.tile([P, FK, rblk_b], l2dt, tag="a")
                    for m in range(FK):
                        ps = psp.tile([P, rblk_b], f32, tag="ps")
                        for k in range(NK1):
                            mm_group(ps, w1, k, m, hb, k, k == 0, k == NK1 - 1,
                                     nsub, dr1)
                        col = s * FK + m
                        nc.scalar.activation(a[:, m], ps[:], Act.Tanh,
                                             bias=b1[:, col:col + 1], scale=s_l1)
                    # layer 2: deriv = tanh(W2.T-chunks @ a + b2)
                    d = dpool.tile([P, FK, rblk_b], bf16, tag="d")
                    for m in range(FK):
                        ps = psp.tile([P, rblk_b], f32, tag="ps")
                        for k in range(NK2):
                            mm_group(ps, w2, k, m, a, k, k == 0, k == NK2 - 1,
                                     nsub, dr2)
                        nc.scalar.activation(d[:, m], ps[:], Act.Tanh,
                                             bias=b2[:, m:m + 1], scale=s_l2)
                    # h' += deriv ; refresh matmul mirror.  Split the fp32
                    # adds across DVE and GpSimd to balance engine load.
                    # For "mix" the final mirror feeds the bf16 gate matmul
                    # instead of another fp8 layer-1 pass.
                    last = s == NUM_STEPS - 1
                    if mode == "mix" and last:
                        hb = hbgp.tile([P, FK, rblk_b], cdt, tag="hbg")
                    else:
                        hb = hbp.tile([P, FK, rblk_b], c1dt, tag="hb")
                    for k in range(FK):
                        eng = nc.vector if (mode not in ("fp8", "mix")
                                            or k < 2) else nc.gpsimd
                        eng.tensor_tensor(h[:, k], h[:, k], d[:, k], Alu.add)
                        if mode == "mix" and last:
                            nc.vector.tensor_copy(hb[:, k], h[:, k])
                        else:
                            mirror(hb[:, k], h[:, k])

                # gate + combine, per output feature chunk
                for m in range(FK):
                    ps = psp.tile([P, rblk_b], f32, tag="ps")
                    for k in range(NKG):
                        mm_group(ps, wg, k, m, xb, k, k == 0, False, nsub, drg)
                    for k in range(NKG):
                        mm_group(ps, wg, NKG + k, m, hb, k, False,
                                 k == NKG - 1, nsub, drg)
                    g = gp.tile([P, rblk_b], f32, tag="g")
                    nc.scalar.activation(g[:], ps[:], Act.Sigmoid,
                                         bias=bg[:, m:m + 1], scale=s_g)
                    # out = x + g * (dt*h' - x)
                    dif = scp.tile([P, rblk_b], f32, tag="sc")
                    nc.vector.scalar_tensor_tensor(
                        dif[:], h[:, m], float(DT), xf[:, m],
                        Alu.mult, Alu.subtract)
                    gd = scp.tile([P, rblk_b], f32, tag="sc")
                    nc.vector.tensor_tensor(gd[:], g[:], dif[:], Alu.mult)
                    ot = scp.tile([P, rblk_b], f32, tag="sc")
                    nc.vector.tensor_tensor(ot[:], xf[:, m], gd[:], Alu.add)
                    nc.sync.dma_start(outTr[:, m, rs:rs + rblk_b], ot[:])

    nc.compile()
    return nc


def _build_v2(npc, rblk=1024, mode=MODE):
    """Dual-stream builder: two row-blocks advance through the ODE steps in
    lockstep so each stream's h-update / fp8-mirror latency hides under the
    sibling stream's matmuls, and each pair's gate work is emitted lazily
    into the first steps of the NEXT pair so the tensor engine never drains
    behind the gate's sigmoid/combine tail."""
    import concourse.mybir as mybir
    import concourse.tile as tile
    from concourse import bacc

    f32 = mybir.dt.float32
    bf16 = mybir.dt.bfloat16
    fp8 = mybir.dt.float8e4
    Act = mybir.ActivationFunctionType
    Alu = mybir.AluOpType
    DR = mybir.MatmulPerfMode.DoubleRow

    cdt = fp8 if mode == "fp8" else bf16          # x-mirror / Wg (gate operands)
    c1dt = fp8 if mode in ("fp8", "mix") else bf16     # h-mirror / W1
    l2dt = fp8 if mode in ("fp8", "fp8l2", "mix") else bf16   # a / W2
    ddt = fp8 if mode == "mix" else bf16          # deriv (adds ~1e-4 rel err)
    dr1 = mode in ("fp8", "mix")
    dr2 = mode in ("fp8", "fp8l2", "mix")
    drg = mode == "fp8"

    nc = bacc.Bacc("TRN2", target_bir_lowering=False, debug=False,
                   num_devices=NCORES)

    hT = nc.dram_tensor("hT", [D, npc], f32, kind="ExternalInput").ap()
    xT = nc.dram_tensor("xT", [D, npc], f32, kind="ExternalInput").ap()
    xTb = nc.dram_tensor("xTb", [D, npc], cdt, kind="ExternalInput").ap()
    w1t = nc.dram_tensor("w1t", [D, D], c1dt, kind="ExternalInput").ap()
    w2t = nc.dram_tensor("w2t", [D, D], l2dt, kind="ExternalInput").ap()
    wgt = nc.dram_tensor("wgt", [2 * D, D], cdt, kind="ExternalInput").ap()
    b1e = nc.dram_tensor("b1e", [P, NUM_STEPS * FK], f32, kind="ExternalInput").ap()
    b2c = nc.dram_tensor("b2c", [P, FK], f32, kind="ExternalInput").ap()
    bgc = nc.dram_tensor("bgc", [P, FK], f32, kind="ExternalInput").ap()
    outT = nc.dram_tensor("outT", [D, npc], f32, kind="ExternalOutput").ap()

    hTr = hT.rearrange("(k p) r -> p k r", p=P)
    xTr = xT.rearrange("(k p) r -> p k r", p=P)
    xTbr = xTb.rearrange("(k p) r -> p k r", p=P)
    outTr = outT.rearrange("(k p) r -> p k r", p=P)

    s_l1 = 1.0 / (AW1 * AH) if dr1 else 1.0
    s_l2 = 1.0 / AW2 if dr2 else 1.0
    s_g = 1.0 / CG if drg else 1.0

    nsub = rblk // MMN
    NPAIR = npc // (2 * rblk)

    with tile.TileContext(nc) as tc:
        with ExitStack() as ctx:
            consts = ctx.enter_context(tc.tile_pool(name="consts", bufs=1))
            hp = ctx.enter_context(tc.tile_pool(name="h", bufs=2))
            hbp = ctx.enter_context(tc.tile_pool(name="hb", bufs=1))
            hbgp = ctx.enter_context(tc.tile_pool(name="hbg", bufs=1))
            apool = ctx.enter_context(tc.tile_pool(name="a", bufs=1))
            dpool = ctx.enter_context(tc.tile_pool(name="d", bufs=2))
            xbp = ctx.enter_context(tc.tile_pool(name="xb", bufs=2))
            xfp = ctx.enter_context(tc.tile_pool(name="xf", bufs=3))
            gp = ctx.enter_context(tc.tile_pool(name="g", bufs=2))
            scp = ctx.enter_context(tc.tile_pool(name="sc", bufs=5))
            psp = ctx.enter_context(tc.tile_pool(name="ps", bufs=4, space="PSUM"))

            w1 = consts.tile([P, FK, D], c1dt)
            nc.sync.dma_start(w1[:], w1t.rearrange("(k p) m -> p k m", p=P))
            w2 = consts.tile([P, FK, D], l2dt)
            nc.sync.dma_start(w2[:], w2t.rearrange("(k p) m -> p k m", p=P))
            wg = consts.tile([P, 2 * FK, D], cdt)
            nc.sync.dma_start(wg[:], wgt.rearrange("(k p) m -> p k m", p=P))
            b1 = consts.tile([P, NUM_STEPS * FK], f32)
            nc.sync.dma_start(b1[:], b1e)
            b2 = consts.tile([P, FK], f32)
            nc.sync.dma_start(b2[:], b2c)
            bg = consts.tile([P, FK], f32)
            nc.sync.dma_start(bg[:], bgc)

            def mm_group(ps_tile, wtile, wk, m, rhs_tile, rk, start, stop, dr):
                for sub in range(nsub):
                    if dr:
                        nc.tensor.matmul(
                            ps_tile[:, sub * MMN:(sub + 1) * MMN],
                            wtile[:, 2 * wk:2 * wk + 2, m * P:(m + 1) * P],
                            rhs_tile[:, 2 * rk:2 * rk + 2,
                                     sub * MMN:(sub + 1) * MMN],
                            start=start, stop=stop, perf_mode=DR,
                        )
                    else:
                        nc.tensor.matmul(
                            ps_tile[:, sub * MMN:(sub + 1) * MMN],
                            wtile[:, wk, m * P:(m + 1) * P],
                            rhs_tile[:, rk, sub * MMN:(sub + 1) * MMN],
                            start=start, stop=stop,
                        )

            NK1 = FK // 2 if dr1 else FK
            NK2 = FK // 2 if dr2 else FK
            NKG = FK // 2 if drg else FK

            def mirror(dst, src, eng):
                if mode in ("fp8", "mix"):
                    eng.tensor_scalar_mul(dst, src, AH)
                else:
                    eng.tensor_copy(dst, src)

            def emit_gate_chunk(st, m):
                # gate + combine for one output feature chunk of one stream
                rs = st["rs"]
                ps = psp.tile([P, rblk], f32, tag="ps", name="psg")
                for k in range(NKG):
                    mm_group(ps, wg, k, m, st["xb"], k, k == 0, False, drg)
                for k in range(NKG):
                    mm_group(ps, wg, NKG + k, m, st["hbg"], k, False,
                             k == NKG - 1, drg)
                g = gp.tile([P, rblk], f32, tag="g", name="g")
                nc.scalar.activation(g[:], ps[:], Act.Sigmoid,
                                     bias=bg[:, m:m + 1], scale=s_g)
                xfc = xfp.tile([P, rblk], f32, tag="xf", name="xfc")
                nc.sync.dma_start(xfc[:], xTr[:, m, rs:rs + rblk])
                # out = x + g * (dt*h' - x)
                dif = scp.tile([P, rblk], f32, tag="sc", name="dif")
                nc.vector.scalar_tensor_tensor(
                    dif[:], st["h"][:, m], float(DT), xfc[:],
                    Alu.mult, Alu.subtract)
                gd = scp.tile([P, rblk], f32, tag="sc", name="gd")
                nc.vector.tensor_tensor(gd[:], g[:], dif[:], Alu.mult)
                ot = scp.tile([P, rblk], f32, tag="sc", name="ot")
                nc.gpsimd.tensor_tensor(ot[:], xfc[:], gd[:], Alu.add)
                nc.sync.dma_start(outTr[:, m, rs:rs + rblk], ot[:])

            def emit_step(st, s):
                si = st["si"]
                h, hb = st["h"], st["hb"]
                a = apool.tile([P, FK, rblk], l2dt, tag=f"a{si}", name="a")
                for m in range(FK):
                    ps = psp.tile([P, rblk], f32, tag="ps", name="ps1")
                    for k in range(NK1):
                        mm_group(ps, w1, k, m, hb, k, k == 0, k == NK1 - 1, dr1)
                    col = s * FK + m
                    nc.scalar.activation(a[:, m], ps[:], Act.Tanh,
                                         bias=b1[:, col:col + 1], scale=s_l1)
                d = dpool.tile([P, FK, rblk], ddt, tag=f"d{si}", name="d")
                for m in range(FK):
                    ps = psp.tile([P, rblk], f32, tag="ps", name="ps2")
                    for k in range(NK2):
                        mm_group(ps, w2, k, m, a, k, k == 0, k == NK2 - 1, dr2)
                    nc.scalar.activation(d[:, m], ps[:], Act.Tanh,
                                         bias=b2[:, m:m + 1], scale=s_l2)
                last = s == NUM_STEPS - 1
                if mode == "mix" and last:
                    nhb = hbgp.tile([P, FK, rblk], cdt, tag=f"hbg{si}",
                                    name="hbg")
                else:
                    nhb = hbp.tile([P, FK, rblk], c1dt, tag=f"hb{si}",
                                   name="hb")
                for k in range(FK):
                    nc.vector.tensor_tensor(h[:, k], h[:, k], d[:, k], Alu.add)
                    eng = nc.vector if k < 2 else nc.gpsimd
                    if mode == "mix" and last:
                        eng.tensor_copy(nhb[:, k], h[:, k])
                    else:
                        mirror(nhb[:, k], h[:, k], eng)
                if mode == "mix" and last:
                    st["hbg"] = nhb
                else:
                    st["hb"] = nhb

            pending = []
            for pi in range(NPAIR):
                sts = []
                for si in range(2):
                    rs = (2 * pi + si) * rblk
                    h = hp.tile([P, FK, rblk], f32, tag=f"h{si}", name="h")
                    for k in range(FK):
                        nc.sync.dma_start(h[:, k], hTr[:, k, rs:rs + rblk])
                    xb = xbp.tile([P, FK, rblk], cdt, tag=f"xb{si}", name="xb")
                    nc.sync.dma_start(xb[:], xTbr[:, :, rs:rs + rblk])
                    hb = hbp.tile([P, FK, rblk], c1dt, tag=f"hb{si}",
                                  name="hb0")
                    for k in range(FK):
                        mirror(hb[:, k], h[:, k],
                               nc.vector if k < 2 else nc.gpsimd)
                    st = dict(si=si, rs=rs, h=h, xb=xb, hb=hb, hbg=None)
                    if mode != "mix":
                        st["hbg"] = hb  # gate reads the last step mirror
                    sts.append(st)
                for s in range(NUM_STEPS):
                    for st in sts:
                        emit_step(st, s)
                        if pending:
                            pending.pop(0)()
                for st in sts:
                    if mode != "mix":
                        st["hbg"] = st["hb"]
                pending = [
                    (lambda st=st, m=m: emit_gate_chunk(st, m))
                    for st in sts for m in range(FK)
                ]
            while pending:
                pending.pop(0)()

    nc.compile()
    return nc


def _get_nc(npc, rblk=1024, mode=MODE, v2=True):
    key = (npc, rblk, mode, v2)
    if key not in _CACHE:
        _CACHE[key] = (_build_v2 if v2 else _build)(npc, rblk, mode)
    return _CACHE[key]


def _fp8_np():
    import concourse.mybir as mybir
    return mybir.dt.np(mybir.dt.float8e4)


def _cast_dt(mode):
    # dtype of the x-mirror / W1 / Wg operands
    return _fp8_np() if mode == "fp8" else ml_dtypes.bfloat16


def _host_prep(W1, b1, W2, b2, Wg, bg, mode=MODE):
    cdt = _cast_dt(mode)
    W1 = np.asarray(W1, np.float32)
    W2 = np.asarray(W2, np.float32)
    Wg = np.asarray(Wg, np.float32)
    b1 = np.asarray(b1, np.float32)
    b2 = np.asarray(b2, np.float32)
    bg = np.asarray(bg, np.float32)

    if mode == "fp8":
        sw1, sw2, swga, swgb = AW1, AW2, AWGA, AWGB
    elif mode == "mix":
        sw1, sw2, swga, swgb = AW1, AW2, 1.0, 1.0
    elif mode == "fp8l2":
        sw1, sw2, swga, swgb = 1.0, AW2, 1.0, 1.0
    else:
        sw1 = sw2 = swga = swgb = 1.0
    c1dt = _fp8_np() if mode in ("fp8", "mix") else cdt
    l2dt = _fp8_np() if mode in ("fp8", "fp8l2", "mix") else cdt

    w1t = np.ascontiguousarray((sw1 * DT * W1[:, :D]).T).astype(c1dt)  # [in, out]
    w2t = np.ascontiguousarray((sw2 * W2).T).astype(l2dt)
    wgt = np.ascontiguousarray(
        np.concatenate([swga * Wg[:, :D].T, swgb * DT * Wg[:, D:].T],
                       axis=0)).astype(cdt)

    ts = (DT * np.arange(NUM_STEPS)).astype(np.float32)
    b1r = b1.reshape(FK, P)                                        # [m, p]
    wtr = np.ascontiguousarray(W1[:, D]).reshape(FK, P)            # [m, p]
    b1e = b1r[None, :, :] + ts[:, None, None] * wtr[None, :, :]    # [s, m, p]
    b1e = np.ascontiguousarray(b1e.transpose(2, 0, 1).reshape(P, NUM_STEPS * FK))
    b2c = np.ascontiguousarray(b2.reshape(FK, P).T)
    bgc = np.ascontiguousarray(bg.reshape(FK, P).T)
    return dict(w1t=w1t, w2t=w2t, wgt=wgt,
                b1e=b1e.astype(np.float32),
                b2c=b2c.astype(np.float32), bgc=bgc.astype(np.float32))


def _make_in_map(x_slice, h_slice, weights, mode=MODE):
    cdt = _cast_dt(mode)
    xs = 1.0 if mode != "fp8" else AX
    xTc = np.ascontiguousarray(x_slice.T)
    return dict(
        hT=np.ascontiguousarray(h_slice.T) * np.float32(1.0 / DT),
        xT=xTc,
        xTb=(xTc * np.float32(xs)).astype(cdt) if mode == "fp8"
        else xTc.astype(cdt),
        **weights,
    )


def kernel(current_node_features, previous_hidden_state, W1, b1, W2, b2, Wg, bg):
    from concourse.bass_utils import run_bass_kernel_spmd

    x = np.asarray(current_node_features, np.float32)
    h0 = np.asarray(previous_hidden_state, np.float32)
    weights = _host_prep(W1, b1, W2, b2, Wg, bg)

    in_maps = []
    for c in range(NCORES):
        sl = slice(c * NPC, (c + 1) * NPC)
        in_maps.append(_make_in_map(x[sl], h0[sl], weights))

    nc = _get_nc(NPC)
    trace = bool(os.environ.get("BASS_TRACE"))
    if trace:
        try:
            import antenv.axon_hooks  # noqa: F401
        except ImportError:
            # no NTFF shim installed (see test.py) -> tracing would crash
            os.environ["BASS_NEVER_TRACE"] = "1"
            trace = False
    res = run_bass_kernel_spmd(nc, in_maps, core_ids=list(range(NCORES)),
                               trace=trace)
    LAST["res"] = res

    out = np.empty((N_TOTAL, D), np.float32)
    for c in range(NCORES):
        out[c * NPC:(c + 1) * NPC] = res.results[c]["outT"].T
    return out, out

